# revision 35
# baseline (speedup 1.0000x reference)
"""LoRA-linear Trainium2 Bass kernel (bf16 I/O, k-streamed prologue).

Computes, for T adapters: out[t] = x @ W.T + (x @ A_t.T) @ B_t.T + bias
Output: [T, B, S, Dout] float32 (device stores bf16; host upcasts).

Sharding: data-parallel over tokens across 8 NeuronCores (2048 tokens/core);
W/bias/selected-LoRA replicated. Matmul inputs are bf16 (host-cast);
accumulation stays fp32; outputs stored bf16 (abs error ~half-ulp(4.5)
≈ 0.008 ≪ the 0.09 budget at rel<2e-2).

Per-core layout puts Dout on PSUM partitions (out.T tiles [dout=128, tok]):
  lowT[32t+j, tok] = sum_d A_t[j,d] x[tok,d]   (PE, k-streamed)
  base.T[m]  = W[m-tile] @ x.T                 (PE, 8 k-tile accumulation)
  delta.T[t,m] = B_t.T row-group matmuls (K=16, tile_position=(32t,0), the
               four adapters issue back-to-back into distinct PSUM banks so
               3 run concurrently per the XBUS budget)
  out.T[t,m] = base.T[m] + delta.T[t,m]        (VectorE tensor_add)

Schedule:
  - Small tensors (A) load first on the sync ring so warm-up matmuls are not
    queued behind the 6 MB of x/W traffic; B/bias ride the scalar ring.
  - Phase A streams k-tiles: as (x_k, w_k) land, the low-rank matmuls and the
    first two base chunks accumulate k-outer, so the DMA prologue is filled
    with real PE work instead of pure warm-up.
  - Main loop per (m, c-chunk): 8 base matmuls -> 2 ScalarE activations
    evacuate base (bias folded) into a duplicated [128,1024] tile -> 4 delta
    matmuls -> 2 VectorE adds of FD=1024 (batching the four FD=512 adds into
    two halves the per-op PSUM overhead and keeps DVE off the critical path).
  - Stores are [128, 2048] bf16 blocks per (t, m), issued as each m finishes.
"""

import sys

if "/opt/trn_rl_repo" not in sys.path:
    sys.path.insert(0, "/opt/trn_rl_repo")

from contextlib import ExitStack

import ml_dtypes
import numpy as np

import concourse.bacc as bacc
import concourse.bass as bass
import concourse.mybir as mybir
import concourse.tile as tile
from concourse import bass_utils

# Problem constants (hardcoded per spec).
B, S, DIN, DOUT, R, NL, T = 4, 4096, 1024, 1024, 16, 8, 4
NCORES = 8
NTOK = B * S                 # 16384
CTOK = NTOK // NCORES        # 2048 tokens per core
KT = DIN // 128              # 8 k-tiles
MT = DOUT // 128             # 8 dout-tiles
NCH = CTOK // 512            # 4 token-chunks of 512

F32 = mybir.dt.float32
BF16 = mybir.dt.bfloat16
NPBF16 = ml_dtypes.bfloat16


def _build_program():
    nc = bacc.Bacc("TRN2", target_bir_lowering=False, debug=False,
                   num_devices=NCORES)

    xt = nc.dram_tensor("xt", [DIN, CTOK], BF16, kind="ExternalInput").ap()
    wt = nc.dram_tensor("wt", [DIN, DOUT], BF16, kind="ExternalInput").ap()
    atp = nc.dram_tensor("atp", [128, KT * 128], BF16, kind="ExternalInput").ap()
    btp = nc.dram_tensor("btp", [128, DOUT], BF16, kind="ExternalInput").ap()
    biasc = nc.dram_tensor("biasc", [128, MT], F32, kind="ExternalInput").ap()
    out = nc.dram_tensor("out", [T, MT, 128, CTOK], BF16,
                         kind="ExternalOutput").ap()

    with tile.TileContext(nc) as tc, ExitStack() as ctx:
        const = ctx.enter_context(tc.tile_pool(name="const", bufs=1))
        brep_sb = ctx.enter_context(tc.tile_pool(name="brep_sb", bufs=4))
        out_sb = ctx.enter_context(tc.tile_pool(name="out_sb", bufs=6))
        bp_ps = ctx.enter_context(tc.tile_pool(name="bp_ps", bufs=2, space="PSUM"))
        dp_ps = ctx.enter_context(tc.tile_pool(name="dp_ps", bufs=3, space="PSUM"))

        # Load order is the prologue schedule: the base stream is gated on
        # W + x chunk 0 (~3 MB), not on the full 6.6 MB of inputs, because
        # the main loop runs chunk-outer and later x chunks stream in behind
        # the compute. x lands as 4 composite chunk DMAs (8x 1 KiB segments
        # per partition) into one (k, c, x)-layout tile.
        x_sb = const.tile([128, KT * CTOK], BF16, tag="xsb")
        xs4 = x_sb.rearrange("p (k c x) -> p k c x", k=KT, c=NCH)
        xt_r = xt.rearrange("(k p) tok -> p k tok", p=128)
        nc.sync.dma_start(xs4[:, :, 0, :], xt_r[:, :, bass.ts(0, 512)])
        wt_t = []
        for k in range(KT):
            tw = const.tile([128, DOUT], BF16, tag=f"wt{k}", name=f"tw{k}")
            nc.sync.dma_start(tw[:], wt[bass.ts(k, 128), :])
            wt_t.append(tw)
        for c in range(1, NCH):
            nc.sync.dma_start(xs4[:, :, c, :], xt_r[:, :, bass.ts(c, 512)])
        at_all = const.tile([128, KT * 128], BF16, tag="at")
        nc.scalar.dma_start(at_all[:], atp[:, :])
        bt_s = const.tile([128, DOUT], BF16, tag="bt")
        nc.scalar.dma_start(bt_s[:], btp[:, :])
        bias_s = const.tile([128, MT], F32, tag="bias")
        nc.scalar.dma_start(bias_s[:], biasc[:, :])
        at_t = [at_all[:, bass.ts(k, 128)] for k in range(KT)]

        lowT_s = const.tile([128, CTOK], BF16, tag="lowT")

        # Warm-up on a memset tile: gates on no DMA, so the PE busy window
        # (HAM un-throttle needs ~3.4us sustained) starts immediately. A
        # second burst after low(c0) bridges to the W-gated base stream.
        wz = const.tile([128, 128], BF16, tag="wz")
        nc.vector.memset(wz[:], 0.0)
        warm = dp_ps.tile([128, 1024], F32, tag="dp", name="warm")
        for _ in range(32):
            nc.tensor.matmul(warm[:, 0:128], wz[:], wz[:],
                             start=True, stop=True)

        def emit_low(c):
            lp = bp_ps.tile([128, 512], F32, tag="bp", name=f"lp{c}")
            for k in range(KT):
                nc.tensor.matmul(lp[:], at_t[k][:], xs4[:, k, c, :],
                                 start=(k == 0), stop=(k == KT - 1))
            nc.vector.tensor_copy(lowT_s[:, bass.ts(c, 512)], lp[:])

        emit_low(0)
        # Second warm-up burst bridges low(c0) to the W-gated base stream so
        # the PE idle never crosses the ~3.4us HAM re-throttle window.
        warm2 = dp_ps.tile([128, 1024], F32, tag="dp", name="warm2")
        for _ in range(45):
            nc.tensor.matmul(warm2[:, 0:128], wz[:], wz[:],
                             start=True, stop=True)

        # Main loop: chunk-outer, m-inner; base(i) is emitted one step ahead
        # of delta(i-1)/adds(i-1) so the PE never head-of-line blocks on PSUM
        # granules still being drained by VectorE.
        mc = [(m, c) for c in range(NCH) for m in range(MT)]
        bps = {}
        breps = {}

        def emit_base(i):
            m, c = mc[i]
            bp = bp_ps.tile([128, 512], F32, tag="bp", name=f"bp{m}_{c}")
            for k in range(KT):
                nc.tensor.matmul(
                    bp[:],
                    wt_t[k][:, bass.ts(m, 128)],
                    xs4[:, k, c, :],
                    start=(k == 0), stop=(k == KT - 1),
                )
            bps[i] = bp
            # Evacuate base twice (duplicated halves) with bias folded in, so
            # the FD=1024 adds read it without a broadcast AP.
            br = brep_sb.tile([128, 1024], F32, tag="brep", name=f"br{m}_{c}")
            for h in range(2):
                nc.scalar.activation(
                    br[:, bass.ts(h, 512)], bps[i][:],
                    mybir.ActivationFunctionType.Identity,
                    bias=bias_s[:, m:m + 1],
                )
            breps[i] = br

        out_r = out.rearrange("t m p x -> p m t x")

        def emit_delta_add(i):
            m, c = mc[i]
            # Per-chunk staging tile [128, t(4) x 512] bf16: both TT writes
            # and the store read are contiguous, and stores drain per chunk
            # instead of bunching at each m boundary.
            om = out_sb.tile([128, T * 512], BF16, tag="om", name=f"om{m}_{c}")
            gA = dp_ps.tile([128, 1024], F32, tag="dp", name=f"gA{m}_{c}")
            gB = dp_ps.tile([128, 1024], F32, tag="dp", name=f"gB{m}_{c}")
            halves = [gA[:, 0:512], gA[:, 512:1024],
                      gB[:, 0:512], gB[:, 512:1024]]
            for t in range(T):
                nc.tensor.matmul(
                    halves[t],
                    bt_s[32 * t:32 * t + R, bass.ts(m, 128)],
                    lowT_s[32 * t:32 * t + R, bass.ts(c, 512)],
                    start=True, stop=True,
                    tile_position=(32 * t, 0),
                )
            om3 = om.rearrange("p (t x) -> p t x", t=T)
            if i == len(mc) - 1:
                # Final iteration: split adds/stores in half so the last
                # store leaves ~1us earlier instead of waiting both FD=1024
                # adds.
                nc.vector.tensor_add(om[:, 0:1024], breps[i][:], gA[:])
                nc.sync.dma_start(out_r[:, m, 0:2, bass.ts(c, 512)],
                                  om3[:, 0:2, :])
                nc.vector.tensor_add(om[:, 1024:2048], breps[i][:], gB[:])
                nc.sync.dma_start(out_r[:, m, 2:4, bass.ts(c, 512)],
                                  om3[:, 2:4, :])
            else:
                nc.vector.tensor_add(om[:, 0:1024], breps[i][:], gA[:])
                nc.vector.tensor_add(om[:, 1024:2048], breps[i][:], gB[:])
                nc.sync.dma_start(out_r[:, m, :, bass.ts(c, 512)], om3)

        # low(c+1) is emitted two iterations before block c+1 begins: the TT
        # pipeline has buffered work to drain while the PE runs the low-rank
        # burst, so VectorE (the pacer) never goes idle at block boundaries.
        for i in range(len(mc) + 1):
            if i < len(mc):
                m, c = mc[i]
                if m == MT - 2 and c < NCH - 1:
                    emit_low(c + 1)
                emit_base(i)
            if i >= 1:
                emit_delta_add(i - 1)

    nc.compile()
    return nc


_NC = None


def _get_program():
    global _NC
    if _NC is None:
        _NC = _build_program()
    return _NC


def kernel(**inputs):
    x = np.ascontiguousarray(np.asarray(inputs["x"], dtype=np.float32))
    W = np.asarray(inputs["W"], dtype=np.float32)
    bias_v = np.asarray(inputs["bias"], dtype=np.float32)
    lora_A = np.asarray(inputs["lora_A"], dtype=np.float32)
    lora_B = np.asarray(inputs["lora_B"], dtype=np.float32)
    tuner_index = np.asarray(inputs["tuner_index"]).astype(np.int64)

    assert x.shape == (B, S, DIN) and W.shape == (DOUT, DIN)
    assert tuner_index.shape == (T,)

    A_sel = lora_A[tuner_index]          # [T, R, Din]
    B_sel = lora_B[tuner_index]          # [T, Dout, R]

    xT = np.ascontiguousarray(x.reshape(NTOK, DIN).T).astype(NPBF16)
    wt = np.ascontiguousarray(W.T).astype(NPBF16)       # [Din, Dout]
    # at_all[p, k*128 + (32t+j)] = A_t[j, k*128+p]: per-partition contiguous
    # so the device load is one clean 2 KiB/partition DMA.
    atp = np.zeros((128, KT, T, 32), NPBF16)
    atp[:, :, :, :R] = A_sel.transpose(2, 0, 1).reshape(KT, 128, T, R) \
                            .transpose(1, 0, 2, 3).astype(NPBF16)
    atp = np.ascontiguousarray(atp.reshape(128, KT * 128))
    btp = np.zeros((128, DOUT), NPBF16)
    btp.reshape(T, 32, DOUT)[:, :R, :] = B_sel.transpose(0, 2, 1).astype(NPBF16)
    biasc = np.ascontiguousarray(bias_v.reshape(MT, 128).T)   # [128, MT]

    in_maps = []
    for c in range(NCORES):
        in_maps.append({
            "xt": np.ascontiguousarray(xT[:, c * CTOK:(c + 1) * CTOK]),
            "wt": wt,
            "atp": atp,
            "btp": btp,
            "biasc": biasc,
        })

    nc = _get_program()
    res = bass_utils.run_bass_kernel_spmd(nc, in_maps, core_ids=list(range(NCORES)))

    big = np.empty((T, MT, 128, NTOK), np.float32)
    for c in range(NCORES):
        big[:, :, :, c * CTOK:(c + 1) * CTOK] = res.results[c]["out"]
    # [T, m, p, tok] -> [T, tok, m*128+p]
    full = np.ascontiguousarray(big.transpose(0, 3, 1, 2))
    return full.reshape(T, B, S, DOUT)


# revision 36
# speedup vs baseline: 1.0048x; 1.0048x over previous
"""LoRA-linear Trainium2 Bass kernel (bf16 I/O, chunk-streamed).

Computes, for T adapters: out[t] = x @ W.T + (x @ A_t.T) @ B_t.T + bias
Output: [T, B, S, Dout] float32 (device stores bf16; host upcasts — bf16
rounding of the output costs ~0.002 rel, far under the 2e-2 gate, and
halves the 32 MB/core of store traffic that bounded the f32 version).

Sharding: data-parallel over tokens across 8 NeuronCores (2048 tokens/core);
W/bias/selected-LoRA replicated. Matmul inputs are bf16 (host-cast);
accumulation stays fp32.

Per-core layout puts Dout on PSUM partitions (out.T tiles [dout=128, tok]):
  lowT[32t+j, tok] = sum_d A_t[j,d] x[tok,d]   (PE, per 512-token chunk)
  base.T[m]  = W[m-tile] @ x.T                 (PE, 8 k-tile accumulation)
  delta.T[t,m] = B_t.T row-group matmuls (K=16, tile_position=(32t,0); the
               four adapters issue back-to-back into distinct PSUM banks so
               they run ~3x concurrent per the XBUS budget)
  out.T[t,m] = base.T[m] + delta.T[t,m]        (VectorE tensor_add)

Schedule (engine balance: PE ~2.3us/chunk is the pacer, DVE ~2.27,
ScalarE ~1.4):
  - Chunk-outer main loop: the base stream gates on W + x-chunk-0 (~3 MB)
    instead of the full 6.6 MB of inputs; later x chunks stream in behind
    the compute (composite per-chunk DMAs into one (k, c, x)-layout tile).
  - Warm-up matmuls on a memset tile (no DMA gate) keep the PE HAM clock
    window busy (~3.4us sustained flips 1.2 -> 2.4 GHz) through the
    prologue; low(c0) runs as soon as x-chunk-0 lands.
  - Per (m, c): 8 base matmuls -> 2 ScalarE activations evacuate base
    (bias folded) into a duplicated [128,1024] tile -> 4 delta matmuls ->
    2 VectorE adds of FD=1024 (batching four FD=512 adds halves the
    per-op PSUM overhead: fp32 PSUM-src tensor_tensor is stuck in 1x mode
    at (120+FD)/0.96GHz) -> one [128, t(4)x512] bf16 store per chunk.
    All TT writes and store reads are contiguous APs (a strided-write
    variant showed a rare HW-timing-dependent corruption).
  - low(c+1) is emitted two iterations early so the TT pipeline has
    buffered work while the PE runs the 1.7us low-rank burst.
  - PSUM: 2x[128,512] base accumulators + 3x[128,1024] delta granules
    fill all 8 banks.
"""

import sys

if "/opt/trn_rl_repo" not in sys.path:
    sys.path.insert(0, "/opt/trn_rl_repo")

from contextlib import ExitStack

import ml_dtypes
import numpy as np

import concourse.bacc as bacc
import concourse.bass as bass
import concourse.mybir as mybir
import concourse.tile as tile
from concourse import bass_utils

# Problem constants (hardcoded per spec).
B, S, DIN, DOUT, R, NL, T = 4, 4096, 1024, 1024, 16, 8, 4
NCORES = 8
NTOK = B * S                 # 16384
CTOK = NTOK // NCORES        # 2048 tokens per core
KT = DIN // 128              # 8 k-tiles
MT = DOUT // 128             # 8 dout-tiles
NCH = CTOK // 512            # 4 token-chunks of 512

F32 = mybir.dt.float32
BF16 = mybir.dt.bfloat16
NPBF16 = ml_dtypes.bfloat16


def _build_program():
    nc = bacc.Bacc("TRN2", target_bir_lowering=False, debug=False,
                   num_devices=NCORES)

    xt = nc.dram_tensor("xt", [DIN, CTOK], BF16, kind="ExternalInput").ap()
    wt = nc.dram_tensor("wt", [DIN, DOUT], BF16, kind="ExternalInput").ap()
    atp = nc.dram_tensor("atp", [128, KT * 128], BF16, kind="ExternalInput").ap()
    btp = nc.dram_tensor("btp", [128, DOUT], BF16, kind="ExternalInput").ap()
    biasc = nc.dram_tensor("biasc", [128, MT], F32, kind="ExternalInput").ap()
    out = nc.dram_tensor("out", [T, MT, 128, CTOK], BF16,
                         kind="ExternalOutput").ap()

    with tile.TileContext(nc) as tc, ExitStack() as ctx:
        const = ctx.enter_context(tc.tile_pool(name="const", bufs=1))
        brep_sb = ctx.enter_context(tc.tile_pool(name="brep_sb", bufs=4))
        out_sb = ctx.enter_context(tc.tile_pool(name="out_sb", bufs=6))
        bp_ps = ctx.enter_context(tc.tile_pool(name="bp_ps", bufs=2, space="PSUM"))
        dp_ps = ctx.enter_context(tc.tile_pool(name="dp_ps", bufs=3, space="PSUM"))

        # Load order is the prologue schedule: the base stream is gated on
        # W + x chunk 0 (~3 MB), not on the full 6.6 MB of inputs, because
        # the main loop runs chunk-outer and later x chunks stream in behind
        # the compute. x lands as 4 composite chunk DMAs (8x 1 KiB segments
        # per partition) into one (k, c, x)-layout tile.
        x_sb = const.tile([128, KT * CTOK], BF16, tag="xsb")
        xs4 = x_sb.rearrange("p (k c x) -> p k c x", k=KT, c=NCH)
        xt_r = xt.rearrange("(k p) tok -> p k tok", p=128)
        nc.sync.dma_start(xs4[:, :, 0, :], xt_r[:, :, bass.ts(0, 512)])
        wt_t = []
        for k in range(KT):
            tw = const.tile([128, DOUT], BF16, tag=f"wt{k}", name=f"tw{k}")
            nc.sync.dma_start(tw[:], wt[bass.ts(k, 128), :])
            wt_t.append(tw)
        for c in range(1, NCH):
            nc.sync.dma_start(xs4[:, :, c, :], xt_r[:, :, bass.ts(c, 512)])
        at_all = const.tile([128, KT * 128], BF16, tag="at")
        nc.scalar.dma_start(at_all[:], atp[:, :])
        bt_s = const.tile([128, DOUT], BF16, tag="bt")
        nc.scalar.dma_start(bt_s[:], btp[:, :])
        bias_s = const.tile([128, MT], F32, tag="bias")
        nc.scalar.dma_start(bias_s[:], biasc[:, :])
        at_t = [at_all[:, bass.ts(k, 128)] for k in range(KT)]

        lowT_s = const.tile([128, CTOK], BF16, tag="lowT")

        # Warm-up on a memset tile: gates on no DMA, so the PE busy window
        # (HAM un-throttle needs ~3.4us sustained) starts immediately. A
        # second burst after low(c0) bridges to the W-gated base stream.
        wz = const.tile([128, 128], BF16, tag="wz")
        nc.vector.memset(wz[:], 0.0)
        warm = dp_ps.tile([128, 1024], F32, tag="dp", name="warm")
        for _ in range(32):
            nc.tensor.matmul(warm[:, 0:128], wz[:], wz[:],
                             start=True, stop=True)

        def emit_low(c):
            lp = bp_ps.tile([128, 512], F32, tag="bp", name=f"lp{c}")
            for k in range(KT):
                nc.tensor.matmul(lp[:], at_t[k][:], xs4[:, k, c, :],
                                 start=(k == 0), stop=(k == KT - 1))
            nc.vector.tensor_copy(lowT_s[:, bass.ts(c, 512)], lp[:])

        emit_low(0)
        # Second warm-up burst bridges low(c0) to the W-gated base stream so
        # the PE idle never crosses the ~3.4us HAM re-throttle window.
        warm2 = dp_ps.tile([128, 1024], F32, tag="dp", name="warm2")
        for _ in range(45):
            nc.tensor.matmul(warm2[:, 0:128], wz[:], wz[:],
                             start=True, stop=True)

        # Main loop: chunk-outer, m-inner; base(i) is emitted one step ahead
        # of delta(i-1)/adds(i-1) so the PE never head-of-line blocks on PSUM
        # granules still being drained by VectorE.
        mc = [(m, c) for c in range(NCH) for m in range(MT)]
        bps = {}
        breps = {}

        def emit_base(i):
            m, c = mc[i]
            bp = bp_ps.tile([128, 512], F32, tag="bp", name=f"bp{m}_{c}")
            for k in range(KT):
                nc.tensor.matmul(
                    bp[:],
                    wt_t[k][:, bass.ts(m, 128)],
                    xs4[:, k, c, :],
                    start=(k == 0), stop=(k == KT - 1),
                )
            bps[i] = bp
            # Evacuate base twice (duplicated halves) with bias folded in, so
            # the FD=1024 adds read it without a broadcast AP.
            br = brep_sb.tile([128, 1024], F32, tag="brep", name=f"br{m}_{c}")
            for h in range(2):
                nc.scalar.activation(
                    br[:, bass.ts(h, 512)], bps[i][:],
                    mybir.ActivationFunctionType.Identity,
                    bias=bias_s[:, m:m + 1],
                )
            breps[i] = br

        out_r = out.rearrange("t m p x -> p m t x")

        def emit_delta_add(i):
            m, c = mc[i]
            # Per-chunk staging tile [128, t(4) x 512] bf16: both TT writes
            # and the store read are contiguous, and stores drain per chunk
            # instead of bunching at each m boundary.
            om = out_sb.tile([128, T * 512], BF16, tag="om", name=f"om{m}_{c}")
            gA = dp_ps.tile([128, 1024], F32, tag="dp", name=f"gA{m}_{c}")
            gB = dp_ps.tile([128, 1024], F32, tag="dp", name=f"gB{m}_{c}")
            halves = [gA[:, 0:512], gA[:, 512:1024],
                      gB[:, 0:512], gB[:, 512:1024]]
            for t in range(T):
                nc.tensor.matmul(
                    halves[t],
                    bt_s[32 * t:32 * t + R, bass.ts(m, 128)],
                    lowT_s[32 * t:32 * t + R, bass.ts(c, 512)],
                    start=True, stop=True,
                    tile_position=(32 * t, 0),
                )
            om3 = om.rearrange("p (t x) -> p t x", t=T)
            if i == len(mc) - 1:
                # Final iteration: split adds/stores in half so the last
                # store leaves ~1us earlier instead of waiting both FD=1024
                # adds.
                nc.vector.tensor_add(om[:, 0:1024], breps[i][:], gA[:])
                nc.sync.dma_start(out_r[:, m, 0:2, bass.ts(c, 512)],
                                  om3[:, 0:2, :])
                nc.vector.tensor_add(om[:, 1024:2048], breps[i][:], gB[:])
                nc.sync.dma_start(out_r[:, m, 2:4, bass.ts(c, 512)],
                                  om3[:, 2:4, :])
            else:
                nc.vector.tensor_add(om[:, 0:1024], breps[i][:], gA[:])
                nc.vector.tensor_add(om[:, 1024:2048], breps[i][:], gB[:])
                nc.sync.dma_start(out_r[:, m, :, bass.ts(c, 512)], om3)

        # low(c+1) is emitted two iterations before block c+1 begins: the TT
        # pipeline has buffered work to drain while the PE runs the low-rank
        # burst, so VectorE (the pacer) never goes idle at block boundaries.
        for i in range(len(mc) + 1):
            if i < len(mc):
                m, c = mc[i]
                if m == MT - 2 and c < NCH - 1:
                    emit_low(c + 1)
                emit_base(i)
            if i >= 1:
                emit_delta_add(i - 1)

    nc.compile()
    return nc


_NC = None


def _get_program():
    global _NC
    if _NC is None:
        _NC = _build_program()
    return _NC


def kernel(**inputs):
    x = np.ascontiguousarray(np.asarray(inputs["x"], dtype=np.float32))
    W = np.asarray(inputs["W"], dtype=np.float32)
    bias_v = np.asarray(inputs["bias"], dtype=np.float32)
    lora_A = np.asarray(inputs["lora_A"], dtype=np.float32)
    lora_B = np.asarray(inputs["lora_B"], dtype=np.float32)
    tuner_index = np.asarray(inputs["tuner_index"]).astype(np.int64)

    assert x.shape == (B, S, DIN) and W.shape == (DOUT, DIN)
    assert tuner_index.shape == (T,)

    A_sel = lora_A[tuner_index]          # [T, R, Din]
    B_sel = lora_B[tuner_index]          # [T, Dout, R]

    xT = np.ascontiguousarray(x.reshape(NTOK, DIN).T).astype(NPBF16)
    wt = np.ascontiguousarray(W.T).astype(NPBF16)       # [Din, Dout]
    # at_all[p, k*128 + (32t+j)] = A_t[j, k*128+p]: per-partition contiguous
    # so the device load is one clean 2 KiB/partition DMA.
    atp = np.zeros((128, KT, T, 32), NPBF16)
    atp[:, :, :, :R] = A_sel.transpose(2, 0, 1).reshape(KT, 128, T, R) \
                            .transpose(1, 0, 2, 3).astype(NPBF16)
    atp = np.ascontiguousarray(atp.reshape(128, KT * 128))
    btp = np.zeros((128, DOUT), NPBF16)
    btp.reshape(T, 32, DOUT)[:, :R, :] = B_sel.transpose(0, 2, 1).astype(NPBF16)
    biasc = np.ascontiguousarray(bias_v.reshape(MT, 128).T)   # [128, MT]

    in_maps = []
    for c in range(NCORES):
        in_maps.append({
            "xt": np.ascontiguousarray(xT[:, c * CTOK:(c + 1) * CTOK]),
            "wt": wt,
            "atp": atp,
            "btp": btp,
            "biasc": biasc,
        })

    nc = _get_program()
    res = bass_utils.run_bass_kernel_spmd(nc, in_maps, core_ids=list(range(NCORES)))

    big = np.empty((T, MT, 128, NTOK), np.float32)
    for c in range(NCORES):
        big[:, :, :, c * CTOK:(c + 1) * CTOK] = res.results[c]["out"]
    # [T, m, p, tok] -> [T, tok, m*128+p]
    full = np.ascontiguousarray(big.transpose(0, 3, 1, 2))
    return full.reshape(T, B, S, DOUT)


# revision 40
# speedup vs baseline: 1.0286x; 1.0238x over previous
"""LoRA-linear Trainium2 Bass kernel (bf16 I/O, chunk-streamed).

Computes, for T adapters: out[t] = x @ W.T + (x @ A_t.T) @ B_t.T + bias
Output: [T, B, S, Dout] float32 (device stores bf16; host upcasts — bf16
rounding of the output costs ~0.002 rel, far under the 2e-2 gate, and
halves the 32 MB/core of store traffic that bounded the f32 version).

Sharding: data-parallel over tokens across 8 NeuronCores (2048 tokens/core);
W/bias/selected-LoRA replicated. Matmul inputs are bf16 (host-cast);
accumulation stays fp32.

Per-core layout puts Dout on PSUM partitions (out.T tiles [dout=128, tok]):
  lowT[32t+j, tok] = sum_d A_t[j,d] x[tok,d]   (PE, per 512-token chunk)
  base.T[m]  = W[m-tile] @ x.T                 (PE, 8 k-tile accumulation)
  delta.T[t,m] = B_t.T row-group matmuls (K=16, tile_position=(32t,0); the
               four adapters issue back-to-back into distinct PSUM banks so
               they run ~3x concurrent per the XBUS budget)
  out.T[t,m] = base.T[m] + delta.T[t,m]        (VectorE tensor_add)

Schedule (engine balance: PE ~2.3us/chunk is the pacer, DVE ~2.27,
ScalarE ~1.4):
  - Chunk-outer main loop: the base stream gates on W + x-chunk-0 (~3 MB)
    instead of the full 6.6 MB of inputs; later x chunks stream in behind
    the compute (composite per-chunk DMAs into one (k, c, x)-layout tile).
  - Warm-up matmuls on a memset tile (no DMA gate) keep the PE HAM clock
    window busy (~3.4us sustained flips 1.2 -> 2.4 GHz) through the
    prologue; low(c0) runs as soon as x-chunk-0 lands.
  - Per (m, c): 8 base matmuls -> 2 ScalarE activations evacuate base
    (bias folded) into a duplicated [128,1024] tile -> 4 delta matmuls ->
    2 VectorE adds of FD=1024 (batching four FD=512 adds halves the
    per-op PSUM overhead: fp32 PSUM-src tensor_tensor is stuck in 1x mode
    at (120+FD)/0.96GHz) -> one [128, t(4)x512] bf16 store per chunk.
    All TT writes and store reads are contiguous APs (a strided-write
    variant showed a rare HW-timing-dependent corruption).
  - low(c+1) is emitted two iterations early so the TT pipeline has
    buffered work while the PE runs the 1.7us low-rank burst.
  - PSUM: 2x[128,512] base accumulators + 3x[128,1024] delta granules
    fill all 8 banks.
"""

import sys

if "/opt/trn_rl_repo" not in sys.path:
    sys.path.insert(0, "/opt/trn_rl_repo")

from contextlib import ExitStack

import ml_dtypes
import numpy as np

import concourse.bacc as bacc
import concourse.bass as bass
import concourse.mybir as mybir
import concourse.tile as tile
from concourse import bass_utils

# Problem constants (hardcoded per spec).
B, S, DIN, DOUT, R, NL, T = 4, 4096, 1024, 1024, 16, 8, 4
NCORES = 8
NTOK = B * S                 # 16384
CTOK = NTOK // NCORES        # 2048 tokens per core
KT = DIN // 128              # 8 k-tiles
MT = DOUT // 128             # 8 dout-tiles
NCH = CTOK // 512            # 4 token-chunks of 512

F32 = mybir.dt.float32
BF16 = mybir.dt.bfloat16
NPBF16 = ml_dtypes.bfloat16


def _build_program():
    nc = bacc.Bacc("TRN2", target_bir_lowering=False, debug=False,
                   num_devices=NCORES)

    xt = nc.dram_tensor("xt", [128, KT * CTOK], BF16, kind="ExternalInput").ap()
    wt = nc.dram_tensor("wt", [DIN, DOUT], BF16, kind="ExternalInput").ap()
    atp = nc.dram_tensor("atp", [128, KT * 128], BF16, kind="ExternalInput").ap()
    btp = nc.dram_tensor("btp", [128, DOUT], BF16, kind="ExternalInput").ap()
    biasc = nc.dram_tensor("biasc", [128, MT], F32, kind="ExternalInput").ap()
    out = nc.dram_tensor("out", [T, MT, 128, CTOK], BF16,
                         kind="ExternalOutput").ap()

    with tile.TileContext(nc) as tc, ExitStack() as ctx:
        const = ctx.enter_context(tc.tile_pool(name="const", bufs=1))
        brep_sb = ctx.enter_context(tc.tile_pool(name="brep_sb", bufs=4))
        out_sb = ctx.enter_context(tc.tile_pool(name="out_sb", bufs=6))
        bp_ps = ctx.enter_context(tc.tile_pool(name="bp_ps", bufs=2, space="PSUM"))
        dp_ps = ctx.enter_context(tc.tile_pool(name="dp_ps", bufs=3, space="PSUM"))

        # Load order is the prologue schedule: the base stream is gated on
        # W + x chunk 0 (~3 MB), not on the full 6.6 MB of inputs, because
        # the main loop runs chunk-outer and later x chunks stream in behind
        # the compute. The host packs x in (c, k, x) order so each chunk is
        # one fully contiguous 1 MB DMA.
        x_sb = const.tile([128, KT * CTOK], BF16, tag="xsb")
        xs4 = x_sb.rearrange("p (c k x) -> p c k x", k=KT, c=NCH)
        CBLK = KT * 512
        nc.sync.dma_start(x_sb[:, 0:CBLK], xt[:, 0:CBLK])
        wt_t = []
        for k in range(KT):
            tw = const.tile([128, DOUT], BF16, tag=f"wt{k}", name=f"tw{k}")
            nc.sync.dma_start(tw[:], wt[bass.ts(k, 128), :])
            wt_t.append(tw)
        for c in range(1, NCH):
            nc.sync.dma_start(x_sb[:, bass.ts(c, CBLK)], xt[:, bass.ts(c, CBLK)])
        at_all = const.tile([128, KT * 128], BF16, tag="at")
        nc.scalar.dma_start(at_all[:], atp[:, :])
        bt_s = const.tile([128, DOUT], BF16, tag="bt")
        nc.scalar.dma_start(bt_s[:], btp[:, :])
        bias_s = const.tile([128, MT], F32, tag="bias")
        nc.scalar.dma_start(bias_s[:], biasc[:, :])
        at_t = [at_all[:, bass.ts(k, 128)] for k in range(KT)]

        lowT_s = const.tile([128, CTOK], BF16, tag="lowT")

        # Warm-up on a memset tile: gates on no DMA, so the PE busy window
        # (HAM un-throttle needs ~3.4us sustained) starts immediately. A
        # second burst after low(c0) bridges to the W-gated base stream.
        wz = const.tile([128, 128], BF16, tag="wz")
        nc.vector.memset(wz[:], 0.0)
        warm = dp_ps.tile([128, 1024], F32, tag="dp", name="warm")
        for _ in range(32):
            nc.tensor.matmul(warm[:, 0:128], wz[:], wz[:],
                             start=True, stop=True)

        def emit_low(c):
            lp = bp_ps.tile([128, 512], F32, tag="bp", name=f"lp{c}")
            for k in range(KT):
                nc.tensor.matmul(lp[:], at_t[k][:], xs4[:, c, k, :],
                                 start=(k == 0), stop=(k == KT - 1))
            nc.vector.tensor_copy(lowT_s[:, bass.ts(c, 512)], lp[:])

        emit_low(0)
        # Second warm-up burst bridges low(c0) to the W-gated base stream so
        # the PE idle never crosses the ~3.4us HAM re-throttle window.
        warm2 = dp_ps.tile([128, 1024], F32, tag="dp", name="warm2")
        for _ in range(45):
            nc.tensor.matmul(warm2[:, 0:128], wz[:], wz[:],
                             start=True, stop=True)

        # Main loop: chunk-outer, m-inner; base(i) is emitted one step ahead
        # of delta(i-1)/adds(i-1) so the PE never head-of-line blocks on PSUM
        # granules still being drained by VectorE.
        mc = [(m, c) for c in range(NCH) for m in range(MT)]
        bps = {}
        breps = {}

        def emit_base(i):
            m, c = mc[i]
            bp = bp_ps.tile([128, 512], F32, tag="bp", name=f"bp{m}_{c}")
            for k in range(KT):
                nc.tensor.matmul(
                    bp[:],
                    wt_t[k][:, bass.ts(m, 128)],
                    xs4[:, c, k, :],
                    start=(k == 0), stop=(k == KT - 1),
                )
            bps[i] = bp
            # Evacuate base twice (duplicated halves) with bias folded in, so
            # the FD=1024 adds read it without a broadcast AP.
            br = brep_sb.tile([128, 1024], F32, tag="brep", name=f"br{m}_{c}")
            for h in range(2):
                nc.scalar.activation(
                    br[:, bass.ts(h, 512)], bps[i][:],
                    mybir.ActivationFunctionType.Identity,
                    bias=bias_s[:, m:m + 1],
                )
            breps[i] = br

        out_r = out.rearrange("t m p x -> p m t x")

        def emit_delta_add(i):
            m, c = mc[i]
            # Per-chunk staging tile [128, t(4) x 512] bf16: both TT writes
            # and the store read are contiguous, and stores drain per chunk
            # instead of bunching at each m boundary.
            om = out_sb.tile([128, T * 512], BF16, tag="om", name=f"om{m}_{c}")
            gA = dp_ps.tile([128, 1024], F32, tag="dp", name=f"gA{m}_{c}")
            gB = dp_ps.tile([128, 1024], F32, tag="dp", name=f"gB{m}_{c}")
            halves = [gA[:, 0:512], gA[:, 512:1024],
                      gB[:, 0:512], gB[:, 512:1024]]
            for t in range(T):
                nc.tensor.matmul(
                    halves[t],
                    bt_s[32 * t:32 * t + R, bass.ts(m, 128)],
                    lowT_s[32 * t:32 * t + R, bass.ts(c, 512)],
                    start=True, stop=True,
                    tile_position=(32 * t, 0),
                )
            om3 = om.rearrange("p (t x) -> p t x", t=T)
            if i == len(mc) - 1:
                # Final iteration: split adds/stores in half so the last
                # store leaves ~1us earlier instead of waiting both FD=1024
                # adds.
                nc.vector.tensor_add(om[:, 0:1024], breps[i][:], gA[:])
                nc.sync.dma_start(out_r[:, m, 0:2, bass.ts(c, 512)],
                                  om3[:, 0:2, :])
                nc.vector.tensor_add(om[:, 1024:2048], breps[i][:], gB[:])
                nc.sync.dma_start(out_r[:, m, 2:4, bass.ts(c, 512)],
                                  om3[:, 2:4, :])
            else:
                nc.vector.tensor_add(om[:, 0:1024], breps[i][:], gA[:])
                nc.vector.tensor_add(om[:, 1024:2048], breps[i][:], gB[:])
                nc.sync.dma_start(out_r[:, m, :, bass.ts(c, 512)], om3)

        # low(c+1) is emitted two iterations before block c+1 begins: the TT
        # pipeline has buffered work to drain while the PE runs the low-rank
        # burst, so VectorE (the pacer) never goes idle at block boundaries.
        for i in range(len(mc) + 1):
            if i < len(mc):
                m, c = mc[i]
                if m == MT - 2 and c < NCH - 1:
                    emit_low(c + 1)
                emit_base(i)
            if i >= 1:
                emit_delta_add(i - 1)

    nc.compile()
    return nc


_NC = None


def _get_program():
    global _NC
    if _NC is None:
        _NC = _build_program()
    return _NC


def kernel(**inputs):
    x = np.ascontiguousarray(np.asarray(inputs["x"], dtype=np.float32))
    W = np.asarray(inputs["W"], dtype=np.float32)
    bias_v = np.asarray(inputs["bias"], dtype=np.float32)
    lora_A = np.asarray(inputs["lora_A"], dtype=np.float32)
    lora_B = np.asarray(inputs["lora_B"], dtype=np.float32)
    tuner_index = np.asarray(inputs["tuner_index"]).astype(np.int64)

    assert x.shape == (B, S, DIN) and W.shape == (DOUT, DIN)
    assert tuner_index.shape == (T,)

    A_sel = lora_A[tuner_index]          # [T, R, Din]
    B_sel = lora_B[tuner_index]          # [T, Dout, R]

    xT = np.ascontiguousarray(x.reshape(NTOK, DIN).T).astype(NPBF16)
    wt = np.ascontiguousarray(W.T).astype(NPBF16)       # [Din, Dout]
    # at_all[p, k*128 + (32t+j)] = A_t[j, k*128+p]: per-partition contiguous
    # so the device load is one clean 2 KiB/partition DMA.
    atp = np.zeros((128, KT, T, 32), NPBF16)
    atp[:, :, :, :R] = A_sel.transpose(2, 0, 1).reshape(KT, 128, T, R) \
                            .transpose(1, 0, 2, 3).astype(NPBF16)
    atp = np.ascontiguousarray(atp.reshape(128, KT * 128))
    btp = np.zeros((128, DOUT), NPBF16)
    btp.reshape(T, 32, DOUT)[:, :R, :] = B_sel.transpose(0, 2, 1).astype(NPBF16)
    biasc = np.ascontiguousarray(bias_v.reshape(MT, 128).T)   # [128, MT]

    in_maps = []
    for c in range(NCORES):
        # xt[p, (chunk, k, x)] = x.T[k*128+p, chunk*512+x]: each 512-token
        # chunk lands as one fully contiguous 1 MB DMA on the device.
        shard = xT[:, c * CTOK:(c + 1) * CTOK]
        xtc = shard.reshape(KT, 128, NCH, 512).transpose(1, 2, 0, 3)
        in_maps.append({
            "xt": np.ascontiguousarray(xtc.reshape(128, KT * CTOK)),
            "wt": wt,
            "atp": atp,
            "btp": btp,
            "biasc": biasc,
        })

    nc = _get_program()
    res = bass_utils.run_bass_kernel_spmd(nc, in_maps, core_ids=list(range(NCORES)))

    big = np.empty((T, MT, 128, NTOK), np.float32)
    for c in range(NCORES):
        big[:, :, :, c * CTOK:(c + 1) * CTOK] = res.results[c]["out"]
    # [T, m, p, tok] -> [T, tok, m*128+p]
    full = np.ascontiguousarray(big.transpose(0, 3, 1, 2))
    return full.reshape(T, B, S, DOUT)
